# revision 1
# baseline (speedup 1.0000x reference)
"""CenterLoss segment-reduce kernel for Trainium2, 8 NeuronCores.

Computes: mean over 1000 classes of sqrt(sum_{i in class} ||x_i - c_class||^2)
for x [65536, 512] f32, labels [65536] int, centers [1000, 512] f32.

Strategy (data-parallel over the batch axis, 8192 samples/core):
  - centers are quantized (fp8-e4m3 by default) host-side; each sample's
    center row is fetched with the gpsimd dma_gather from the DRAM table.
    Gather order is chosen so gathered rows land in the same (partition,
    tile) slot as the x rows, which lets the x load use one fully
    contiguous 16KB descriptor per partition.
  - per chunk: diff = x - g (DVE, mixed f32/fp8, fp16 out); per tile:
    sq = sum(diff^2) (ACT Square with free-dim accumulator).
  - per-class segment sum of sq via one-hot trick: oh = (iota == label) * sq
    (DVE tensor_scalar two-op, fp16 4x mode), then a ones-vector matmul
    column-sums oh into a [1, 1024] PSUM accumulator (PE).
  - per-class partials are AllReduced across the 8 cores, then sqrt + sum +
    scale on device; every core emits the same scalar.
"""

import os
import sys

for _p in (
    "/opt/trn_rl_repo",
    os.path.expanduser("~/.axon_site/_ro/trn_rl_repo"),
):
    if os.path.isdir(_p) and _p not in sys.path:
        sys.path.insert(0, _p)

import numpy as np

from concourse import bacc, bass_utils, mybir, tile

dt = mybir.dt

NCORES = 8
N, D, NCLS = 65536, 512, 1000
NCLS_PAD = 1024
NS = N // NCORES        # samples per core
TT = NS // 128          # 128-sample tiles per core

# Tunables (read at build time; _in_maps must agree on CH).
CH = 1024               # samples per gather/DMA chunk
GDT = "float8e4"        # gathered-centers dtype (float8e4 | float16)
XBUFS = 3               # x/gather pipeline depth
WBUFS = 6               # per-tile work pipeline depth
SCRATCH = 65536         # SWDGE descriptor ring bytes
SUBCHUNK = False        # one subtract per chunk instead of per tile
DIFFDT = "float16"      # diff dtype
NSWQ = 1                # SWDGE queues; gathers alternate across them
SPKT = True             # dma_gather single_packet (False breaks layout)
VARCH = None            # optional explicit chunk-size list (sums to NS)
OHMODE = "scaled"       # "scaled": oh=(iota==lab)*sq, ones stationary
                        # "sqstat": oh=(iota==lab), sq (fp16) stationary
XCHM = 1                # x-DMA chunk = XCHM consecutive gather chunks
TSENG = "dve"           # one-hot tensor_scalar engine: dve | gpsimd | alt
OHDT = "float16"        # iota/one-hot dtype (float16 | float32 | bfloat16)

AF = mybir.ActivationFunctionType
ALU = mybir.AluOpType

_cache = {}


def _build(repeat=1, collective=True, ablate=(), hwloop=False,
           tail_in_loop=False, dynloop=False):
    """Build the Bass program. repeat>1 re-runs the main loop (for timing);
    the final scale keeps the output correct (per-class sums scale by
    `repeat`, so sqrt sums scale by sqrt(repeat)). ablate: subset of
    {"gather","xload","sub","act","onehot","mm"} — skip stages for
    cost-model ablation (output becomes wrong)."""
    key = (repeat, collective, tuple(sorted(ablate)), CH, GDT, XBUFS, WBUFS,
           SUBCHUNK, DIFFDT, hwloop, NSWQ, SPKT, tuple(VARCH or ()),
           tail_in_loop, OHMODE, dynloop, XCHM, TSENG, OHDT)
    if key in _cache:
        return _cache[key]
    ab = set(ablate)
    chunks = list(VARCH) if VARCH else [CH] * (NS // CH)
    assert sum(chunks) == NS and all(c % 128 == 0 for c in chunks)
    starts = [sum(chunks[:i]) for i in range(len(chunks))]
    nchunk = len(chunks)
    nc = bacc.Bacc(
        "TRN2", target_bir_lowering=False, debug=False, num_devices=NCORES,
        dynamic_dma_scratch_size=SCRATCH, num_swdge_queues=NSWQ,
    )
    gdt = getattr(dt, GDT)
    x = nc.dram_tensor("x", [NS, D], dt.float32, kind="ExternalInput").ap()
    c16 = nc.dram_tensor("c16", [NCLS, D], gdt, kind="ExternalInput").ap()
    idx = nc.dram_tensor("idx", [128, NS // 16], dt.int16, kind="ExternalInput").ap()
    labf = nc.dram_tensor("labf", [128, TT], dt.float32, kind="ExternalInput").ap()
    if OHMODE == "factored":
        labhi = nc.dram_tensor(
            "labhi", [128, TT], dt.float32, kind="ExternalInput"
        ).ap()
    ohdt = getattr(dt, OHDT)
    iota = nc.dram_tensor("iota", [128, NCLS_PAD], ohdt, kind="ExternalInput").ap()
    ones = nc.dram_tensor("ones", [128, 1], ohdt, kind="ExternalInput").ap()
    out = nc.dram_tensor("out", [1, 1], dt.float32, kind="ExternalOutput").ap()
    if dynloop:
        hwloop = True
        rcount = nc.dram_tensor(
            "rcount", [1, 1], dt.uint32, kind="ExternalInput"
        ).ap()

    with tile.TileContext(nc) as tc:
        with (
            tc.tile_pool(name="const", bufs=1) as cpool,
            tc.tile_pool(name="xs", bufs=XBUFS) as xpool,
            tc.tile_pool(name="gs", bufs=XBUFS) as gpool,
            tc.tile_pool(name="diffp", bufs=2) as dpool_sb,
            tc.tile_pool(name="work", bufs=WBUFS) as wpool,
            tc.tile_pool(name="psum", bufs=1, space="PSUM") as ppool,
            tc.tile_pool(name="dram", bufs=1, space="DRAM") as dpool,
        ):
            if OHMODE == "factored":
                iota_t = cpool.tile([128, 128], ohdt)
                iotah_t = cpool.tile([128, 8], ohdt)
                labhi_t = cpool.tile([128, TT], dt.float32)
                nc.sync.dma_start(
                    out=iotah_t[:], in_=iota[:, 0:8]
                )
                nc.sync.dma_start(out=labhi_t[:], in_=labhi)
            else:
                iota_t = cpool.tile([128, NCLS_PAD], ohdt)
            ones_t = cpool.tile([128, 1], ohdt)
            labf_t = cpool.tile([128, TT], dt.float32)
            idx_t = cpool.tile([128, NS // 16], dt.int16)
            nc.sync.dma_start(
                out=iota_t[:],
                in_=iota[:, 0:128] if OHMODE == "factored" else iota,
            )
            nc.sync.dma_start(out=ones_t[:], in_=ones)
            nc.sync.dma_start(out=labf_t[:], in_=labf)
            nc.sync.dma_start(out=idx_t[:], in_=idx)

            if "mm" not in ab:
                if OHMODE == "factored":
                    psf = ppool.tile([128, 8], dt.float32)
                else:
                    ps0 = ppool.tile([1, 512], dt.float32)
                    ps1 = ppool.tile([1, 512], dt.float32)
                if hwloop and not tail_in_loop:
                    if OHMODE == "factored":
                        nc.vector.memset(psf[:], 0.0)
                    else:
                        nc.vector.memset(ps0[:], 0.0)
                        nc.vector.memset(ps1[:], 0.0)

            def _tail():
                scale = 1.0 / (NCLS * (1 if tail_in_loop else repeat) ** 0.5)
                pshape = [128, 8] if OHMODE == "factored" else [1, NCLS_PAD]
                part = cpool.tile(pshape, dt.float32, tag="part")
                if "mm" not in ab:
                    if OHMODE == "factored":
                        nc.scalar.copy(out=part[:], in_=psf[:])
                    else:
                        nc.scalar.copy(out=part[:, 0:512], in_=ps0[:])
                        nc.scalar.copy(out=part[:, 512:NCLS_PAD], in_=ps1[:])
                else:
                    nc.vector.memset(part[:], 1.0)
                if collective:
                    cc_in = dpool.tile(pshape, dt.float32, tag="cci")
                    cc_out = dpool.tile(pshape, dt.float32, tag="cco")
                    nc.sync.dma_start(out=cc_in[:], in_=part[:])
                    nc.gpsimd.collective_compute(
                        "AllReduce",
                        ALU.add,
                        replica_groups=[list(range(NCORES))],
                        ins=[cc_in.opt()],
                        outs=[cc_out.opt()],
                    )
                    red = cpool.tile(pshape, dt.float32, tag="red")
                    nc.sync.dma_start(out=red[:], in_=cc_out[:])
                else:
                    red = part
                rt = cpool.tile(pshape, dt.float32, tag="rt")
                nc.scalar.activation(out=rt[:], in_=red[:], func=AF.Sqrt)
                res = cpool.tile([1, 1], dt.float32, tag="res")
                if OHMODE == "factored":
                    rsum = cpool.tile([128, 1], dt.float32, tag="rsum")
                    nc.vector.tensor_reduce(
                        out=rsum[:], in_=rt[:], axis=mybir.AxisListType.X,
                        op=ALU.add,
                    )
                    rsum16 = cpool.tile([128, 1], ohdt, tag="rsum16")
                    nc.vector.tensor_copy(rsum16[:], rsum[:])
                    pst = ppool.tile([1, 1], dt.float32, tag="pst")
                    nc.tensor.matmul(
                        out=pst[:], lhsT=rsum16[:], rhs=ones_t[:],
                        start=True, stop=True,
                    )
                    nc.scalar.mul(out=res[:], in_=pst[:], mul=scale)
                else:
                    tot = cpool.tile([1, 1], dt.float32, tag="tot")
                    nc.vector.tensor_reduce(
                        out=tot[:], in_=rt[:], axis=mybir.AxisListType.X,
                        op=ALU.add,
                    )
                    nc.scalar.mul(out=res[:], in_=tot[:], mul=scale)
                nc.sync.dma_start(out=out, in_=res[:])

            import contextlib
            if dynloop:
                rc_t = cpool.tile([1, 1], dt.uint32)
                nc.sync.dma_start(out=rc_t[:], in_=rcount)
                rv = nc.values_load(rc_t[:], min_val=0, max_val=1 << 20,
                                    skip_runtime_bounds_check=True)
                loop_ctx = tc.For_i(0, rv, 1)
            else:
                loop_ctx = (
                    tc.For_i(0, repeat, 1) if hwloop and repeat > 1
                    else contextlib.nullcontext()
                )
            unrolled = 1 if hwloop else repeat
            with loop_ctx:
              for j in range(nchunk * unrolled):
                  rep, j = divmod(j, nchunk)
                  if tail_in_loop and j == 0 and "mm" not in ab:
                      if OHMODE == "factored":
                          nc.vector.memset(psf[:], 0.0)
                      else:
                          nc.vector.memset(ps0[:], 0.0)
                          nc.vector.memset(ps1[:], 0.0)
                  ch, r0 = chunks[j], starts[j]
                  tpc = ch // 128
                  if j % XCHM == 0:
                      xch = sum(chunks[j : j + XCHM])
                      xs_w = xpool.tile(
                          [128, xch // 128, D], dt.float32, tag="xs"
                      )
                      # row-block layout per gather chunk: partition p holds
                      # rows r0 + p*tpc .. r0 + p*tpc + tpc-1 -> contiguous
                      # per-partition runs, one DMA covering XCHM chunks.
                      if "xload" not in ab:
                          if XCHM == 1:
                              xsrc = x[r0 : r0 + xch, :].rearrange(
                                  "(p t) d -> p t d", p=128
                              )
                              nc.sync.dma_start(out=xs_w[:], in_=xsrc)
                          else:
                              # each sub-chunk keeps its own row-block wrap
                              for jj in range(XCHM):
                                  cj, rj = chunks[j + jj], starts[j + jj]
                                  tj = cj // 128
                                  off = (
                                      sum(chunks[j : j + jj]) // 128
                                  )
                                  xsrc = x[rj : rj + cj, :].rearrange(
                                      "(p t) d -> p t d", p=128
                                  )
                                  nc.sync.dma_start(
                                      out=xs_w[:, off : off + tj, :],
                                      in_=xsrc,
                                  )
                      xs_off = 0
                  else:
                      xs_off += chunks[j - 1] // 128
                  xs = xs_w[:, xs_off : xs_off + tpc, :]
                  gs = gpool.tile([128, tpc, D], gdt, tag="gs")
                  if "gather" not in ab:
                      nc.gpsimd.dma_gather(
                          out_ap=gs[:],
                          in_ap=c16,
                          idxs_ap=idx_t[:, r0 // 16 : (r0 + ch) // 16],
                          num_idxs=ch,
                          num_idxs_reg=ch,
                          elem_size=D,
                          queue_num=j % NSWQ,
                          single_packet=SPKT,
                      )
                  x_in = gs[:] if "xload" in ab else xs
                  g_in = xs if "gather" in ab else gs[:]
                  ddt = getattr(dt, DIFFDT)
                  if "sub" in ab:
                      d_in = x_in
                  elif SUBCHUNK:
                      diff = dpool_sb.tile([128, tpc, D], ddt, tag="diff")
                      nc.vector.tensor_tensor(
                          out=diff[:], in0=x_in[:], in1=g_in[:], op=ALU.subtract
                      )
                      d_in = diff
                  else:
                      d_in = None
                  for t in range(tpc):
                      T = r0 // 128 + t
                      first = (not hwloop) and rep == 0 and T == 0
                      last = (not hwloop) and rep == unrolled - 1 and T == TT - 1
                      if d_in is None:
                          dtl = wpool.tile([128, D], ddt, tag="difft")
                          nc.vector.tensor_tensor(
                              out=dtl[:], in0=x_in[:, t, :], in1=g_in[:, t, :],
                              op=ALU.subtract,
                          )
                          d_slice = dtl[:]
                      else:
                          d_slice = d_in[:, t, :]
                      if "act" not in ab:
                          scr = wpool.tile([128, D], gdt, tag="scr")
                          sq = wpool.tile([128, 1], dt.float32, tag="sq")
                          nc.scalar.activation(
                              out=scr[:], in_=d_slice, func=AF.Square,
                              accum_out=sq[:],
                          )
                          sq_in = sq[:]
                      else:
                          sq_in = labf_t[:, T : T + 1]
                      if OHMODE == "factored":
                          if "onehot" not in ab:
                              ohlo = wpool.tile([128, 128], ohdt, tag="ohlo")
                              ohhi = wpool.tile([128, 8], ohdt, tag="ohhi")
                              nc.vector.tensor_scalar(
                                  out=ohlo[:], in0=iota_t[:],
                                  scalar1=labf_t[:, T : T + 1], scalar2=None,
                                  op0=ALU.is_equal,
                              )
                              nc.vector.tensor_scalar(
                                  out=ohhi[:], in0=iotah_t[:],
                                  scalar1=labhi_t[:, T : T + 1], scalar2=sq_in,
                                  op0=ALU.is_equal, op1=ALU.mult,
                              )
                          if "mm" not in ab:
                              nc.tensor.matmul(
                                  out=psf[:], lhsT=ohlo[:], rhs=ohhi[:],
                                  start=first, stop=last,
                                  skip_group_check=hwloop,
                              )
                          continue
                      if "onehot" not in ab:
                          oh = wpool.tile([128, NCLS_PAD], ohdt, tag="oh")
                          ts_eng = (
                              nc.gpsimd if TSENG == "gpsimd"
                              or (TSENG == "alt" and T % 2) else nc.vector
                          )
                          if OHMODE == "sqstat":
                              ts_eng.tensor_scalar(
                                  out=oh[:], in0=iota_t[:],
                                  scalar1=labf_t[:, T : T + 1], scalar2=None,
                                  op0=ALU.is_equal,
                              )
                          else:
                              ts_eng.tensor_scalar(
                                  out=oh[:], in0=iota_t[:],
                                  scalar1=labf_t[:, T : T + 1], scalar2=sq_in,
                                  op0=ALU.is_equal, op1=ALU.mult,
                              )
                          oh_in = oh
                      else:
                          oh_in = iota_t
                      if OHMODE == "sqstat" and "act" not in ab:
                          sq16 = wpool.tile([128, 1], ohdt, tag="sq16")
                          nc.vector.tensor_copy(sq16[:], sq_in)
                          stat = sq16
                      else:
                          stat = ones_t
                      if "mm" not in ab:
                          nc.tensor.matmul(
                              out=ps0[:], lhsT=stat[:], rhs=oh_in[:, 0:512],
                              start=first, stop=last,
                              skip_group_check=hwloop,
                          )
                          nc.tensor.matmul(
                              out=ps1[:], lhsT=stat[:], rhs=oh_in[:, 512:NCLS_PAD],
                              start=first, stop=last,
                              skip_group_check=hwloop,
                          )

                  if tail_in_loop and j == nchunk - 1:
                      _tail()
            if not tail_in_loop:
                _tail()

    nc.compile()
    _cache[key] = nc
    return nc


def _in_maps(x, labels, centers):
    x = np.ascontiguousarray(np.asarray(x), dtype=np.float32)
    labels = np.asarray(labels).astype(np.int64)
    centers_q = np.asarray(centers).astype(mybir.dt.np(getattr(dt, GDT)))
    ohnp = mybir.dt.np(getattr(dt, OHDT))
    iota = np.ascontiguousarray(
        np.broadcast_to(np.arange(NCLS_PAD, dtype=ohnp), (128, NCLS_PAD))
    )
    ones = np.ones((128, 1), ohnp)
    chunks = list(VARCH) if VARCH else [CH] * (NS // CH)
    starts = [sum(chunks[:i]) for i in range(len(chunks))]
    maps = []
    for k in range(NCORES):
        lk = labels[k * NS : (k + 1) * NS]
        # row-block order per chunk: sample at (partition p, tile t of chunk
        # j) is lk[r0 + p*tpc + t]; gather index i of chunk j must be
        # lk[r0 + (i%128)*tpc + i//128]; labf[p, r0//128 + t] = that label.
        idx16 = np.empty((16, NS // 16), np.int16)
        labf = np.empty((128, TT), np.float32)
        for ch, r0 in zip(chunks, starts):
            tpc = ch // 128
            lkc = lk[r0 : r0 + ch].reshape(128, tpc)     # [p, t]
            idx_lin = lkc.T.reshape(ch)                  # [i = t*128 + p]
            idx16[:, r0 // 16 : (r0 + ch) // 16] = idx_lin.astype(
                np.int16
            ).reshape(ch // 16, 16).T
            labf[:, r0 // 128 : (r0 + ch) // 128] = lkc.astype(np.float32)
        idx16 = np.ascontiguousarray(np.tile(idx16, (8, 1)))
        labhi = None
        if OHMODE == "factored":
            labhi = np.ascontiguousarray(np.floor_divide(labf, 128.0)).astype(
                np.float32
            )
            labf = np.ascontiguousarray(np.mod(labf, 128.0)).astype(np.float32)
        m = {
            "x": np.ascontiguousarray(x[k * NS : (k + 1) * NS]),
            "c16": centers_q,
            "idx": idx16,
            "labf": labf,
            "iota": iota,
            "ones": ones,
        }
        if labhi is not None:
            m["labhi"] = labhi
        maps.append(m)
    return maps


def kernel(x, labels, centers, _trace=False, _repeat=1, **run_kwargs):
    nc = _build(repeat=_repeat)
    maps = _in_maps(x, labels, centers)
    res = bass_utils.run_bass_kernel_spmd(
        nc, maps, list(range(NCORES)), trace=_trace, **run_kwargs
    )
    val = np.float32(res.results[0]["out"].reshape(())[()])
    if _trace:
        kernel.last_result = res
    return np.asarray(val, dtype=np.float32)



# revision 16
# speedup vs baseline: 1.0331x; 1.0331x over previous
"""CenterLoss segment-reduce kernel for Trainium2, 8 NeuronCores.

Computes: mean over 1000 classes of sqrt(sum_{i in class} ||x_i - c_class||^2)
for x [65536, 512] f32, labels [65536] int, centers [1000, 512] f32.

Strategy (data-parallel over the batch axis, 8192 samples/core):
  - x is host-cast to fp16 (halves HBM traffic + enables 2x DVE modes);
    samples are host-sorted by class within each core shard (the result is
    permutation-invariant) so gather rows walk HBM nearly sequentially.
  - centers are quantized (fp8-e4m3) host-side; each sample's center row is
    fetched with the gpsimd dma_gather from the DRAM table. Gather order is
    chosen so gathered rows land in the same (partition, tile) slot as the
    x rows, which lets the x load use one contiguous descriptor/partition.
  - per tile: diff = x - g (DVE); sq = sum(diff^2) (ACT Square with
    free-dim accumulator).
  - per-class segment sum of sq via FACTORED one-hots (class = hi*128+lo):
    ohlo[128,128] = (iota==lab%128), ohhi[128,8] = (iota==lab//128)*sq
    (two small DVE tensor_scalars), then one PE matmul accumulates
    psf[128,8] += ohlo^T @ ohhi across all tiles — ~7x cheaper than the
    [128,1024] one-hot + ones-matmul formulation.
  - per-class partials are AllReduced across the 8 cores, then sqrt + sum +
    scale on device; every core emits the same scalar. The 1/repeat timing
    correction is folded into the sqrt input scale to keep the fp16 cast
    in the factored tail from overflowing at large repeat counts.

Measured (per-iteration of on-device x1025 repeat loop, main loop):
  118.3 us original -> 104.4 us this config. Ablations: SWDGE gather is
  the wall (71.7 us alone at 1 queue; 32.6 us at 4 queues, but extra
  SWDGE queues slow the FULL kernel via descriptor-ring/DVE contention);
  x-load 20.7 us; factored one-hots 22.2 us. tensor_tensor_reduce hangs
  the HW (SQACT path disabled at 64).
"""

import os
import sys

for _p in (
    "/opt/trn_rl_repo",
    os.path.expanduser("~/.axon_site/_ro/trn_rl_repo"),
):
    if os.path.isdir(_p) and _p not in sys.path:
        sys.path.insert(0, _p)

import numpy as np

from concourse import bacc, bass_utils, mybir, tile

dt = mybir.dt

NCORES = 8
N, D, NCLS = 65536, 512, 1000
NCLS_PAD = 1024
NS = N // NCORES        # samples per core
TT = NS // 128          # 128-sample tiles per core

# Tunables (read at build time; _in_maps must agree on CH).
CH = 1024               # samples per gather/DMA chunk
GDT = "float8e4"        # gathered-centers dtype (float8e4 | float16)
XDT = "float16"         # x dtype on device (host-cast; float32|float16|bfloat16)
SORT = True             # host-sort samples by class per core (gather locality)
XBUFS = 3               # x/gather pipeline depth
WBUFS = 6               # per-tile work pipeline depth
SCRATCH = 65536         # SWDGE descriptor ring bytes
SUBCHUNK = False        # one subtract per chunk instead of per tile
DIFFDT = "float16"      # diff dtype
NSWQ = 1                # SWDGE queues; gathers alternate across them
SPKT = True             # dma_gather single_packet (False breaks layout)
VARCH = None            # optional explicit chunk-size list (sums to NS)
OHMODE = "factored"     # "scaled": oh=(iota==lab)*sq, ones stationary
                        # "sqstat": oh=(iota==lab), sq (fp16) stationary
                        # "factored": ohlo[128]xohhi[8] via psf[128,8]
XCHM = 1                # x-DMA chunk = XCHM consecutive gather chunks
TSENG = "dve"           # one-hot tensor_scalar engine: dve | gpsimd | alt
OHDT = "float16"        # iota/one-hot dtype (float16 | float32 | bfloat16)
SQACT = 64              # of 64 tiles: first SQACT use ACT square, rest DVE ttr

# env overrides for experiments: K_<NAME>=value (int, or literal string)
for _name in ("CH", "GDT", "XDT", "SORT", "XBUFS", "WBUFS", "SUBCHUNK",
              "DIFFDT", "NSWQ", "SPKT", "OHMODE", "XCHM", "TSENG", "OHDT",
              "SQACT"):
    _v = os.environ.get("K_" + _name)
    if _v is not None:
        try:
            _v = int(_v)
        except ValueError:
            pass
        globals()[_name] = _v

AF = mybir.ActivationFunctionType
ALU = mybir.AluOpType

_cache = {}


def _build(repeat=1, collective=True, ablate=(), hwloop=False,
           tail_in_loop=False, dynloop=False):
    """Build the Bass program. repeat>1 re-runs the main loop (for timing);
    the final scale keeps the output correct (per-class sums scale by
    `repeat`, so sqrt sums scale by sqrt(repeat)). ablate: subset of
    {"gather","xload","sub","act","onehot","mm"} — skip stages for
    cost-model ablation (output becomes wrong)."""
    key = (repeat, collective, tuple(sorted(ablate)), CH, GDT, XBUFS, WBUFS,
           SUBCHUNK, DIFFDT, hwloop, NSWQ, SPKT, tuple(VARCH or ()),
           tail_in_loop, OHMODE, dynloop, XCHM, TSENG, OHDT, XDT, SQACT)
    if key in _cache:
        return _cache[key]
    ab = set(ablate)
    chunks = list(VARCH) if VARCH else [CH] * (NS // CH)
    assert sum(chunks) == NS and all(c % 128 == 0 for c in chunks)
    starts = [sum(chunks[:i]) for i in range(len(chunks))]
    nchunk = len(chunks)
    nc = bacc.Bacc(
        "TRN2", target_bir_lowering=False, debug=False, num_devices=NCORES,
        dynamic_dma_scratch_size=SCRATCH, num_swdge_queues=NSWQ,
    )
    gdt = getattr(dt, GDT)
    xdt = getattr(dt, XDT)
    x = nc.dram_tensor("x", [NS, D], xdt, kind="ExternalInput").ap()
    c16 = nc.dram_tensor("c16", [NCLS, D], gdt, kind="ExternalInput").ap()
    idx = nc.dram_tensor("idx", [128, NS // 16], dt.int16, kind="ExternalInput").ap()
    labf = nc.dram_tensor("labf", [128, TT], dt.float32, kind="ExternalInput").ap()
    if OHMODE == "factored":
        labhi = nc.dram_tensor(
            "labhi", [128, TT], dt.float32, kind="ExternalInput"
        ).ap()
    ohdt = getattr(dt, OHDT)
    iota = nc.dram_tensor("iota", [128, NCLS_PAD], ohdt, kind="ExternalInput").ap()
    ones = nc.dram_tensor("ones", [128, 1], ohdt, kind="ExternalInput").ap()
    out = nc.dram_tensor("out", [1, 1], dt.float32, kind="ExternalOutput").ap()
    if dynloop:
        hwloop = True
        rcount = nc.dram_tensor(
            "rcount", [1, 1], dt.uint32, kind="ExternalInput"
        ).ap()

    with tile.TileContext(nc) as tc:
        with (
            tc.tile_pool(name="const", bufs=1) as cpool,
            tc.tile_pool(name="xs", bufs=XBUFS) as xpool,
            tc.tile_pool(name="gs", bufs=XBUFS) as gpool,
            tc.tile_pool(name="diffp", bufs=2) as dpool_sb,
            tc.tile_pool(name="work", bufs=WBUFS) as wpool,
            tc.tile_pool(name="psum", bufs=1, space="PSUM") as ppool,
            tc.tile_pool(name="dram", bufs=1, space="DRAM") as dpool,
        ):
            if OHMODE == "factored":
                iota_t = cpool.tile([128, 128], ohdt)
                iotah_t = cpool.tile([128, 8], ohdt)
                labhi_t = cpool.tile([128, TT], dt.float32)
                nc.sync.dma_start(
                    out=iotah_t[:], in_=iota[:, 0:8]
                )
                nc.sync.dma_start(out=labhi_t[:], in_=labhi)
            else:
                iota_t = cpool.tile([128, NCLS_PAD], ohdt)
            ones_t = cpool.tile([128, 1], ohdt)
            labf_t = cpool.tile([128, TT], dt.float32)
            idx_t = cpool.tile([128, NS // 16], dt.int16)
            nc.sync.dma_start(
                out=iota_t[:],
                in_=iota[:, 0:128] if OHMODE == "factored" else iota,
            )
            nc.sync.dma_start(out=ones_t[:], in_=ones)
            nc.sync.dma_start(out=labf_t[:], in_=labf)
            nc.sync.dma_start(out=idx_t[:], in_=idx)

            if "mm" not in ab:
                if OHMODE == "factored":
                    psf = ppool.tile([128, 8], dt.float32)
                else:
                    ps0 = ppool.tile([1, 512], dt.float32)
                    ps1 = ppool.tile([1, 512], dt.float32)
                if hwloop and not tail_in_loop:
                    if OHMODE == "factored":
                        nc.vector.memset(psf[:], 0.0)
                    else:
                        nc.vector.memset(ps0[:], 0.0)
                        nc.vector.memset(ps1[:], 0.0)

            def _tail():
                rep_eff = 1 if tail_in_loop else repeat
                scale = 1.0 / (NCLS * rep_eff**0.5)
                pshape = [128, 8] if OHMODE == "factored" else [1, NCLS_PAD]
                part = cpool.tile(pshape, dt.float32, tag="part")
                if "mm" not in ab:
                    if OHMODE == "factored":
                        nc.scalar.copy(out=part[:], in_=psf[:])
                    else:
                        nc.scalar.copy(out=part[:, 0:512], in_=ps0[:])
                        nc.scalar.copy(out=part[:, 512:NCLS_PAD], in_=ps1[:])
                else:
                    nc.vector.memset(part[:], 1.0)
                if collective:
                    cc_in = dpool.tile(pshape, dt.float32, tag="cci")
                    cc_out = dpool.tile(pshape, dt.float32, tag="cco")
                    nc.sync.dma_start(out=cc_in[:], in_=part[:])
                    nc.gpsimd.collective_compute(
                        "AllReduce",
                        ALU.add,
                        replica_groups=[list(range(NCORES))],
                        ins=[cc_in.opt()],
                        outs=[cc_out.opt()],
                    )
                    red = cpool.tile(pshape, dt.float32, tag="red")
                    nc.sync.dma_start(out=red[:], in_=cc_out[:])
                else:
                    red = part
                rt = cpool.tile(pshape, dt.float32, tag="rt")
                # fold the repeat correction into sqrt's input scale:
                # sqrt(red/rep) = sqrt(red)/sqrt(rep) — keeps the factored
                # rsum16 fp16 cast in range for large repeat counts.
                nc.scalar.activation(
                    out=rt[:], in_=red[:], func=AF.Sqrt, scale=1.0 / rep_eff
                )
                res = cpool.tile([1, 1], dt.float32, tag="res")
                if OHMODE == "factored":
                    rsum = cpool.tile([128, 1], dt.float32, tag="rsum")
                    nc.vector.tensor_reduce(
                        out=rsum[:], in_=rt[:], axis=mybir.AxisListType.X,
                        op=ALU.add,
                    )
                    rsum16 = cpool.tile([128, 1], ohdt, tag="rsum16")
                    nc.vector.tensor_copy(rsum16[:], rsum[:])
                    pst = ppool.tile([1, 1], dt.float32, tag="pst")
                    nc.tensor.matmul(
                        out=pst[:], lhsT=rsum16[:], rhs=ones_t[:],
                        start=True, stop=True,
                    )
                    nc.scalar.mul(out=res[:], in_=pst[:], mul=1.0 / NCLS)
                else:
                    tot = cpool.tile([1, 1], dt.float32, tag="tot")
                    nc.vector.tensor_reduce(
                        out=tot[:], in_=rt[:], axis=mybir.AxisListType.X,
                        op=ALU.add,
                    )
                    nc.scalar.mul(out=res[:], in_=tot[:], mul=1.0 / NCLS)
                nc.sync.dma_start(out=out, in_=res[:])

            import contextlib
            if dynloop:
                rc_t = cpool.tile([1, 1], dt.uint32)
                nc.sync.dma_start(out=rc_t[:], in_=rcount)
                rv = nc.values_load(rc_t[:], min_val=0, max_val=1 << 20,
                                    skip_runtime_bounds_check=True)
                loop_ctx = tc.For_i(0, rv, 1)
            else:
                loop_ctx = (
                    tc.For_i(0, repeat, 1) if hwloop and repeat > 1
                    else contextlib.nullcontext()
                )
            unrolled = 1 if hwloop else repeat
            with loop_ctx:
              for j in range(nchunk * unrolled):
                  rep, j = divmod(j, nchunk)
                  if tail_in_loop and j == 0 and "mm" not in ab:
                      if OHMODE == "factored":
                          nc.vector.memset(psf[:], 0.0)
                      else:
                          nc.vector.memset(ps0[:], 0.0)
                          nc.vector.memset(ps1[:], 0.0)
                  ch, r0 = chunks[j], starts[j]
                  tpc = ch // 128
                  if j % XCHM == 0:
                      xch = sum(chunks[j : j + XCHM])
                      xs_w = xpool.tile(
                          [128, xch // 128, D], xdt, tag="xs"
                      )
                      # row-block layout per gather chunk: partition p holds
                      # rows r0 + p*tpc .. r0 + p*tpc + tpc-1 -> contiguous
                      # per-partition runs, one DMA covering XCHM chunks.
                      if "xload" not in ab:
                          if XCHM == 1:
                              xsrc = x[r0 : r0 + xch, :].rearrange(
                                  "(p t) d -> p t d", p=128
                              )
                              nc.sync.dma_start(out=xs_w[:], in_=xsrc)
                          else:
                              # each sub-chunk keeps its own row-block wrap
                              for jj in range(XCHM):
                                  cj, rj = chunks[j + jj], starts[j + jj]
                                  tj = cj // 128
                                  off = (
                                      sum(chunks[j : j + jj]) // 128
                                  )
                                  xsrc = x[rj : rj + cj, :].rearrange(
                                      "(p t) d -> p t d", p=128
                                  )
                                  nc.sync.dma_start(
                                      out=xs_w[:, off : off + tj, :],
                                      in_=xsrc,
                                  )
                      xs_off = 0
                  else:
                      xs_off += chunks[j - 1] // 128
                  xs = xs_w[:, xs_off : xs_off + tpc, :]
                  gs = gpool.tile([128, tpc, D], gdt, tag="gs")
                  if "gather" not in ab:
                      nc.gpsimd.dma_gather(
                          out_ap=gs[:],
                          in_ap=c16,
                          idxs_ap=idx_t[:, r0 // 16 : (r0 + ch) // 16],
                          num_idxs=ch,
                          num_idxs_reg=ch,
                          elem_size=D,
                          queue_num=j % NSWQ,
                          single_packet=SPKT,
                      )
                  x_in = gs[:] if "xload" in ab else xs
                  g_in = xs if "gather" in ab else gs[:]
                  ddt = getattr(dt, DIFFDT)
                  if "sub" in ab:
                      d_in = x_in
                  elif SUBCHUNK:
                      diff = dpool_sb.tile([128, tpc, D], ddt, tag="diff")
                      nc.vector.tensor_tensor(
                          out=diff[:], in0=x_in[:], in1=g_in[:], op=ALU.subtract
                      )
                      d_in = diff
                  else:
                      d_in = None
                  for t in range(tpc):
                      T = r0 // 128 + t
                      first = (not hwloop) and rep == 0 and T == 0
                      last = (not hwloop) and rep == unrolled - 1 and T == TT - 1
                      if d_in is None:
                          dtl = wpool.tile([128, D], ddt, tag="difft")
                          nc.vector.tensor_tensor(
                              out=dtl[:], in0=x_in[:, t, :], in1=g_in[:, t, :],
                              op=ALU.subtract,
                          )
                          d_slice = dtl[:]
                      else:
                          d_slice = d_in[:, t, :]
                      if "act" not in ab:
                          scr = wpool.tile([128, D], gdt, tag="scr")
                          sq = wpool.tile([128, 1], dt.float32, tag="sq")
                          if T < SQACT:
                              nc.scalar.activation(
                                  out=scr[:], in_=d_slice, func=AF.Square,
                                  accum_out=sq[:],
                              )
                          else:
                              nc.vector.tensor_tensor_reduce(
                                  out=scr[:], in0=d_slice, in1=d_slice,
                                  scale=1.0, scalar=0.0,
                                  op0=ALU.mult, op1=ALU.add, accum_out=sq[:],
                              )
                          sq_in = sq[:]
                      else:
                          sq_in = labf_t[:, T : T + 1]
                      if OHMODE == "factored":
                          if "onehot" not in ab:
                              ohlo = wpool.tile([128, 128], ohdt, tag="ohlo")
                              ohhi = wpool.tile([128, 8], ohdt, tag="ohhi")
                              nc.vector.tensor_scalar(
                                  out=ohlo[:], in0=iota_t[:],
                                  scalar1=labf_t[:, T : T + 1], scalar2=None,
                                  op0=ALU.is_equal,
                              )
                              nc.vector.tensor_scalar(
                                  out=ohhi[:], in0=iotah_t[:],
                                  scalar1=labhi_t[:, T : T + 1], scalar2=sq_in,
                                  op0=ALU.is_equal, op1=ALU.mult,
                              )
                          if "mm" not in ab:
                              nc.tensor.matmul(
                                  out=psf[:], lhsT=ohlo[:], rhs=ohhi[:],
                                  start=first, stop=last,
                                  skip_group_check=hwloop,
                              )
                          continue
                      if "onehot" not in ab:
                          oh = wpool.tile([128, NCLS_PAD], ohdt, tag="oh")
                          ts_eng = (
                              nc.gpsimd if TSENG == "gpsimd"
                              or (TSENG == "alt" and T % 2) else nc.vector
                          )
                          if OHMODE == "sqstat":
                              ts_eng.tensor_scalar(
                                  out=oh[:], in0=iota_t[:],
                                  scalar1=labf_t[:, T : T + 1], scalar2=None,
                                  op0=ALU.is_equal,
                              )
                          else:
                              ts_eng.tensor_scalar(
                                  out=oh[:], in0=iota_t[:],
                                  scalar1=labf_t[:, T : T + 1], scalar2=sq_in,
                                  op0=ALU.is_equal, op1=ALU.mult,
                              )
                          oh_in = oh
                      else:
                          oh_in = iota_t
                      if OHMODE == "sqstat" and "act" not in ab:
                          sq16 = wpool.tile([128, 1], ohdt, tag="sq16")
                          nc.vector.tensor_copy(sq16[:], sq_in)
                          stat = sq16
                      else:
                          stat = ones_t
                      if "mm" not in ab:
                          nc.tensor.matmul(
                              out=ps0[:], lhsT=stat[:], rhs=oh_in[:, 0:512],
                              start=first, stop=last,
                              skip_group_check=hwloop,
                          )
                          nc.tensor.matmul(
                              out=ps1[:], lhsT=stat[:], rhs=oh_in[:, 512:NCLS_PAD],
                              start=first, stop=last,
                              skip_group_check=hwloop,
                          )

                  if tail_in_loop and j == nchunk - 1:
                      _tail()
            if not tail_in_loop:
                _tail()

    nc.compile()
    _cache[key] = nc
    return nc


def _in_maps(x, labels, centers):
    xnp = mybir.dt.np(getattr(dt, XDT))
    x = np.ascontiguousarray(np.asarray(x)).astype(xnp)
    labels = np.asarray(labels).astype(np.int64)
    centers_q = np.asarray(centers).astype(mybir.dt.np(getattr(dt, GDT)))
    ohnp = mybir.dt.np(getattr(dt, OHDT))
    iota = np.ascontiguousarray(
        np.broadcast_to(np.arange(NCLS_PAD, dtype=ohnp), (128, NCLS_PAD))
    )
    ones = np.ones((128, 1), ohnp)
    chunks = list(VARCH) if VARCH else [CH] * (NS // CH)
    starts = [sum(chunks[:i]) for i in range(len(chunks))]
    maps = []
    for k in range(NCORES):
        lk = labels[k * NS : (k + 1) * NS]
        xk = x[k * NS : (k + 1) * NS]
        if SORT:
            # class-sort the shard: the result is permutation-invariant and
            # sorted labels make the gather walk HBM nearly sequentially.
            perm = np.argsort(lk, kind="stable")
            lk = lk[perm]
            xk = np.ascontiguousarray(xk[perm])
        # row-block order per chunk: sample at (partition p, tile t of chunk
        # j) is lk[r0 + p*tpc + t]; gather index i of chunk j must be
        # lk[r0 + (i%128)*tpc + i//128]; labf[p, r0//128 + t] = that label.
        idx16 = np.empty((16, NS // 16), np.int16)
        labf = np.empty((128, TT), np.float32)
        for ch, r0 in zip(chunks, starts):
            tpc = ch // 128
            lkc = lk[r0 : r0 + ch].reshape(128, tpc)     # [p, t]
            idx_lin = lkc.T.reshape(ch)                  # [i = t*128 + p]
            idx16[:, r0 // 16 : (r0 + ch) // 16] = idx_lin.astype(
                np.int16
            ).reshape(ch // 16, 16).T
            labf[:, r0 // 128 : (r0 + ch) // 128] = lkc.astype(np.float32)
        idx16 = np.ascontiguousarray(np.tile(idx16, (8, 1)))
        labhi = None
        if OHMODE == "factored":
            labhi = np.ascontiguousarray(np.floor_divide(labf, 128.0)).astype(
                np.float32
            )
            labf = np.ascontiguousarray(np.mod(labf, 128.0)).astype(np.float32)
        m = {
            "x": np.ascontiguousarray(xk),
            "c16": centers_q,
            "idx": idx16,
            "labf": labf,
            "iota": iota,
            "ones": ones,
        }
        if labhi is not None:
            m["labhi"] = labhi
        maps.append(m)
    return maps


def kernel(x, labels, centers, _trace=False, _repeat=1, **run_kwargs):
    nc = _build(repeat=_repeat)
    maps = _in_maps(x, labels, centers)
    res = bass_utils.run_bass_kernel_spmd(
        nc, maps, list(range(NCORES)), trace=_trace, **run_kwargs
    )
    val = np.float32(res.results[0]["out"].reshape(())[()])
    if _trace:
        kernel.last_result = res
    return np.asarray(val, dtype=np.float32)

